# revision 15
# baseline (speedup 1.0000x reference)
"""DeepSeek-V2-Lite-style MoE layer on 8 Trainium2 NeuronCores.

Expert-parallel with load-aware placement: each core owns 8 of the 64 routed
experts, assigned on the host so that every core gets at most BIG_PER_CORE
"big" experts (expected load > the 128-slot capacity tier).  Per-core slot
layout is a fixed pattern of BIG_PER_CORE x 256-capacity + 5 x 128-capacity
expert buffers (11 x 128-row tiles vs 16 for uniform capacity 256).  The fp32
router + grouped-top-k run replicated on every core; local-expert bookkeeping
(which global expert sits in which local slot) is pure per-core *data* (one-hot
masks), so all cores run one SPMD program.

Expert GEMMs run in bf16 activations x float8_e3m4 weights (mixed-dtype PE
matmul, fp32 PSUM accumulation).  Routing weights are applied per-slot at the
down-projection output, and each expert tile is combined by an indirect
scatter-ADD DMA straight into a bf16 [T, H] accumulator initialized by the
shared-expert MLP (tensor-parallel along its intermediate dim).  A bf16
ReduceScatter then sums partials across cores; core c returns output rows
[128c, 128(c+1)).

Config (DeepSeek-V2-Lite): T=1024 H=2048 E=64 K=6 I=1024 G=8 TG=3 C=256
"""
import numpy as np
from contextlib import ExitStack

import concourse.bass as bass
import concourse.tile as tile
from concourse import bacc
from concourse import bass_isa
from concourse import mybir
from concourse import bass_utils

T, H, E, K, I, G, TG, C = 1024, 2048, 64, 6, 1024, 8, 3, 256
S = 2048                  # full shared intermediate (NS * I)
SL = S // 8               # per-core shared slice (TP)
RSF = 2.5
NCORES = 8
EL = E // NCORES          # local experts per core
P = 128
NT = T // P               # token tiles
NKH = H // P              # K-tiles over H
BIG = 1.0e30
W13_SCALE = 64.0          # fp8 weight scale for w13
W2_SCALE = 64.0           # fp8 weight scale for w2
# y_psum = a_sb @ (W2_SCALE*w2) where a_sb = W13_SCALE^2 * a_true
UNSCALE = 1.0 / (W13_SCALE * W13_SCALE * W2_SCALE)

BIG_PER_CORE = 3          # 256-capacity slots per core; rest are 128
CAPS = [256] * BIG_PER_CORE + [128] * (EL - BIG_PER_CORE)
BASES = np.concatenate([[0], np.cumsum(CAPS)]).astype(int)
NSLOT = int(BASES[-1])    # 1408

F32 = mybir.dt.float32
U32 = mybir.dt.uint32
BF16 = mybir.dt.bfloat16
E3M4 = mybir.dt.float8e3
U8 = mybir.dt.uint8
AX = mybir.AxisListType
ALU = mybir.AluOpType
ACTF = mybir.ActivationFunctionType


def build_program(caps, use_collective=True, debug_dump=False):
    nslot = int(np.sum(caps))
    bases = np.concatenate([[0], np.cumsum(caps)]).astype(int)
    nc = bacc.Bacc("TRN2", target_bir_lowering=False, debug=False,
                   num_devices=NCORES)

    # ---- inputs (replicated unless noted)
    hs_bf = nc.dram_tensor("hs_bf", [T, H], BF16, kind="ExternalInput")
    hsT = nc.dram_tensor("hsT", [H, T], F32, kind="ExternalInput")
    hsT_bf = nc.dram_tensor("hsT_bf", [H, T], BF16, kind="ExternalInput")
    gwT = nc.dram_tensor("gwT", [H, E], F32, kind="ExternalInput")
    bias_bc = nc.dram_tensor("bias_bc", [P, E], F32, kind="ExternalInput")
    u128 = nc.dram_tensor("u128", [P, P], F32, kind="ExternalInput")
    ones1 = nc.dram_tensor("ones1", [1, P], F32, kind="ExternalInput")
    onesc = nc.dram_tensor("onesc", [P, 1], F32, kind="ExternalInput")
    ident_b = nc.dram_tensor("ident_b", [P, P], BF16, kind="ExternalInput")
    # per-core: global expert index of each of the 8 local slots, broadcast
    le_bc = nc.dram_tensor("le_bc", [P, EL], F32, kind="ExternalInput")
    # per-core expert weights, fp8 e3m4, pre-scaled:
    # w13_loc[l, ih, kt] = [128, I] chunk (ih: gate/up half, kt: H/128)
    w13_loc = nc.dram_tensor("w13_loc", [EL, 2, NKH, P, I], E3M4,
                             kind="ExternalInput")
    # w2_loc[l, hq, j] = [128, 512] chunk (hq: H/512, j: I/128)
    w2_loc = nc.dram_tensor("w2_loc", [EL, 4, I // P, P, 512], E3M4,
                            kind="ExternalInput")
    sgu_loc = nc.dram_tensor("sgu_loc", [H, 2 * SL], BF16, kind="ExternalInput")
    sd_loc = nc.dram_tensor("sd_loc", [SL, H], BF16, kind="ExternalInput")

    out_t = nc.dram_tensor("out", [P if use_collective else T, H], F32,
                           kind="ExternalOutput")

    # ---- internal DRAM scratch
    tkind = "ExternalOutput" if debug_dump else "Internal"
    table = nc.dram_tensor("tbl_out" if debug_dump else "table",
                           [nslot, 1], U32, kind=tkind)
    # scatter-side table: padding slots point at the dead sink row T
    table_s = nc.dram_tensor("table_s", [nslot, 1], U32, kind="Internal")
    wks_d = nc.dram_tensor("wks_out" if debug_dump else "wks_d",
                           [nslot, 1], F32, kind=tkind)
    cc_acc = nc.dram_tensor("cc_acc", [T + P, H], BF16, kind="Internal")
    if use_collective:
        cc_out = nc.dram_tensor("cc_out", [P, H], BF16, kind="Internal")

    with tile.TileContext(nc) as tc, ExitStack() as ctx:
        cst = ctx.enter_context(tc.tile_pool(name="cst", bufs=1))
        rt = ctx.enter_context(tc.tile_pool(name="rt", bufs=2))
        # expert weight pools open for the whole kernel so their DMAs can
        # prefetch during routing/shared phases
        we_pool = ctx.enter_context(tc.tile_pool(name="we", bufs=4))
        w2_pool = ctx.enter_context(tc.tile_pool(name="w2", bufs=4))

        # ---------------- constants
        bias_t = cst.tile([P, E], F32, tag="bias")
        nc.sync.dma_start(bias_t[:], bias_bc[:])
        u_t = cst.tile([P, P], F32, tag="u128")
        nc.sync.dma_start(u_t[:], u128[:])
        ones_t = cst.tile([1, P], F32, tag="ones1")
        nc.sync.dma_start(ones_t[:], ones1[:])
        onesc_t = cst.tile([P, 1], F32, tag="onesc")
        nc.sync.dma_start(onesc_t[:], onesc[:])
        id_t = cst.tile([P, P], BF16, tag="ident")
        nc.sync.dma_start(id_t[:], ident_b[:])
        le_t = cst.tile([P, EL], F32, tag="le")
        nc.sync.dma_start(le_t[:], le_bc[:])
        gw_t = cst.tile([P, NKH * E], F32, tag="gwT")   # [128, kt*64]
        nc.sync.dma_start(gw_t[:].rearrange("p (kt e) -> p kt e", kt=NKH),
                          gwT[:].rearrange("(kt p) e -> p kt e", p=P))
        iot_e = cst.tile([P, E], F32, tag="iote")
        nc.gpsimd.iota(iot_e[:], pattern=[[1, E]], base=0, channel_multiplier=0,
                       allow_small_or_imprecise_dtypes=True)
        # per-slot within-expert position index (variable caps)
        c_iota = cst.tile([P, nslot], F32, tag="c_iota")
        for l in range(EL):
            nc.gpsimd.iota(c_iota[:, int(bases[l]):int(bases[l + 1])],
                           pattern=[[1, int(caps[l])]], base=0,
                           channel_multiplier=0,
                           allow_small_or_imprecise_dtypes=True)
        # mask64[p, (l, e)] = (e == le[l]) for the 8 local slots
        mask64 = cst.tile([P, EL * E], F32, tag="mask64")
        nc.vector.tensor_tensor(
            mask64[:].rearrange("p (l e) -> p l e", l=EL),
            iot_e[:].rearrange("p (o e) -> p o e", o=1).broadcast_to((P, EL, E)),
            le_t[:].rearrange("p (l o) -> p l o", o=1).broadcast_to((P, EL, E)),
            ALU.is_equal)
        off_sb = cst.tile([1, E], F32, tag="off")
        nc.vector.memset(off_sb[:], 0.0)

        # ============ phase R: routing
        with ExitStack() as r_ctx:
            hst_pool = r_ctx.enter_context(tc.tile_pool(name="hst", bufs=4))
            lg_ps_pool = r_ctx.enter_context(
                tc.tile_pool(name="lgps", bufs=1, space="PSUM"))
            pc_ps_pool = r_ctx.enter_context(
                tc.tile_pool(name="pcps", bufs=1, space="PSUM"))
            tbl_ps_pool = r_ctx.enter_context(
                tc.tile_pool(name="tblps", bufs=1, space="PSUM"))
            wks_ps_pool = r_ctx.enter_context(
                tc.tile_pool(name="wksps", bufs=1, space="PSUM"))

            tbl_ps = tbl_ps_pool.tile([1, nslot], F32, tag="tbl")
            wks_ps = wks_ps_pool.tile([1, nslot], F32, tag="wks")
            nch = (nslot + 511) // 512

            for it in range(NT):
                # -------- router matmul (fp32)
                lg = lg_ps_pool.tile([P, E], F32, tag="logits")
                for kt in range(NKH):
                    hv = hst_pool.tile([P, P], F32, tag="hstr")
                    nc.sync.dma_start(
                        hv[:], hsT[kt * P:(kt + 1) * P, it * P:(it + 1) * P])
                    nc.tensor.matmul(
                        lg[:], hv[:], gw_t[:, kt * E:(kt + 1) * E],
                        start=(kt == 0), stop=(kt == NKH - 1))

                scores = rt.tile([P, E], F32, tag="scores")
                nc.scalar.activation(scores[:], lg[:], ACTF.Sigmoid)
                sc = rt.tile([P, E], F32, tag="sc")
                nc.vector.tensor_tensor(sc[:], scores[:], bias_t[:], ALU.add)

                # -------- grouped top-k
                grp = sc[:].rearrange("p (g e) -> p g e", g=G)
                m1 = rt.tile([P, G], F32, tag="m1")
                nc.vector.tensor_reduce(m1[:], grp, axis=AX.X, op=ALU.max)
                eq = rt.tile([P, E], F32, tag="eq")
                m1b = m1[:].rearrange("p (g o) -> p g o", o=1).broadcast_to(
                    (P, G, G))
                nc.vector.tensor_tensor(
                    eq[:].rearrange("p (g e) -> p g e", g=G), grp, m1b,
                    ALU.is_ge)
                pen = rt.tile([P, E], F32, tag="pen")
                nc.vector.tensor_scalar(pen[:], eq[:], -BIG, None, op0=ALU.mult)
                msk2 = rt.tile([P, E], F32, tag="msk2")
                nc.vector.tensor_tensor(msk2[:], sc[:], pen[:], ALU.add)
                m2 = rt.tile([P, G], F32, tag="m2")
                nc.vector.tensor_reduce(
                    m2[:], msk2[:].rearrange("p (g e) -> p g e", g=G),
                    axis=AX.X, op=ALU.max)
                g2 = rt.tile([P, G], F32, tag="g2")
                nc.vector.tensor_tensor(g2[:], m1[:], m2[:], ALU.add)

                gv8 = rt.tile([P, 8], F32, tag="gv8")
                gi8 = rt.tile([P, 8], U32, tag="gi8")
                nc.vector.max_with_indices(gv8[:], gi8[:], g2[:])
                gmask = rt.tile([P, G], F32, tag="gmask")
                nc.vector.tensor_tensor(gmask[:], g2[:],
                                        gv8[:, 2:3].broadcast_to((P, G)),
                                        ALU.is_ge)
                gm64 = rt.tile([P, E], U8, tag="gm64")
                gmb = gmask[:].rearrange("p (g o) -> p g o", o=1).broadcast_to(
                    (P, G, G))
                nc.vector.tensor_copy(
                    gm64[:].rearrange("p (g e) -> p g e", g=G), gmb)
                scm = rt.tile([P, E], F32, tag="scm")
                nc.vector.memset(scm[:], -BIG)
                nc.vector.copy_predicated(scm[:], gm64[:], sc[:])
                v8 = rt.tile([P, 8], F32, tag="v8")
                i8 = rt.tile([P, 8], U32, tag="i8")
                nc.vector.max_with_indices(v8[:], i8[:], scm[:])

                # one-hots for the 6 chosen experts: ohs[p, (k, e)]
                i8f = rt.tile([P, K], F32, tag="i8f")
                nc.vector.tensor_copy(i8f[:], i8[:, 0:K])
                ohs = rt.tile([P, K * E], F32, tag="ohs")
                nc.vector.tensor_tensor(
                    ohs[:].rearrange("p (k e) -> p k e", k=K),
                    iot_e[:].rearrange("p (o e) -> p o e", o=1)
                    .broadcast_to((P, K, E)),
                    i8f[:].rearrange("p (k o) -> p k o", o=1)
                    .broadcast_to((P, K, E)),
                    ALU.is_equal)
                A = rt.tile([P, E], F32, tag="A")
                nc.vector.tensor_reduce(
                    A[:], ohs[:].rearrange("p (k e) -> p e k", k=K),
                    axis=AX.X, op=ALU.add)
                # scok[p, k] = score of k-th pick
                tmp6 = rt.tile([P, K * E], F32, tag="tmp6")
                nc.vector.tensor_tensor(
                    tmp6[:].rearrange("p (k e) -> p k e", k=K),
                    ohs[:].rearrange("p (k e) -> p k e", k=K),
                    scores[:].rearrange("p (o e) -> p o e", o=1)
                    .broadcast_to((P, K, E)),
                    ALU.mult)
                scok = rt.tile([P, K], F32, tag="scok")
                nc.vector.tensor_reduce(
                    scok[:], tmp6[:].rearrange("p (k e) -> p k e", k=K),
                    axis=AX.X, op=ALU.add)

                # renormalized weights (scaled by RSF and the fp8 unscale)
                ssum = rt.tile([P, 1], F32, tag="ssum")
                nc.vector.tensor_reduce(ssum[:], scok[:], axis=AX.X, op=ALU.add)
                nc.vector.tensor_scalar(ssum[:], ssum[:], 1e-20, None,
                                        op0=ALU.add)
                sinv = rt.tile([P, 1], F32, tag="sinv")
                nc.vector.reciprocal(sinv[:], ssum[:])
                nc.vector.tensor_scalar(sinv[:], sinv[:], RSF * UNSCALE, None,
                                        op0=ALU.mult)
                wk = rt.tile([P, K], F32, tag="wk")
                nc.vector.tensor_scalar(wk[:], scok[:], sinv[:], None,
                                        op0=ALU.mult)
                # W64[p, e] = weight of expert e for this token (0 if unused)
                nc.vector.tensor_tensor(
                    tmp6[:].rearrange("p (k e) -> p k e", k=K),
                    ohs[:].rearrange("p (k e) -> p k e", k=K),
                    wk[:].rearrange("p (k o) -> p k o", o=1)
                    .broadcast_to((P, K, E)),
                    ALU.mult)
                W64 = rt.tile([P, E], F32, tag="W64")
                nc.vector.tensor_reduce(
                    W64[:], tmp6[:].rearrange("p (k e) -> p e k", k=K),
                    axis=AX.X, op=ALU.add)

                # running cumsum over token tiles: pos[p, e]; the per-tile
                # column totals are pos[last] + A[last] (exclusive cumsum)
                pc_ps = pc_ps_pool.tile([P, E], F32, tag="pcps")
                nc.tensor.matmul(pc_ps[:], u_t[:], A[:], start=True,
                                 stop=False)
                nc.tensor.matmul(pc_ps[:], ones_t[:], off_sb[:],
                                 start=False, stop=True)
                pos = rt.tile([P, E], F32, tag="pos")
                nc.vector.tensor_copy(pos[:], pc_ps[:])
                # offset += per-tile column totals (gpsimd partition reduce)
                csar = rt.tile([P, E], F32, tag="csar")
                nc.gpsimd.partition_all_reduce(csar[:], A[:], channels=P,
                                               reduce_op=bass_isa.ReduceOp.add)
                nc.vector.tensor_tensor(off_sb[:], off_sb[:], csar[0:1, :],
                                        ALU.add)

                # local-slot views: posL/AL/WL [p, l] via mask64
                tmp8 = rt.tile([P, EL * E], F32, tag="tmp8")
                posL = rt.tile([P, EL], F32, tag="posL")
                AL = rt.tile([P, EL], F32, tag="AL")
                WL = rt.tile([P, EL], F32, tag="WL")
                for src, dst in ((pos, posL), (A, AL), (W64, WL)):
                    nc.vector.tensor_tensor(
                        tmp8[:].rearrange("p (l e) -> p l e", l=EL),
                        mask64[:].rearrange("p (l e) -> p l e", l=EL),
                        src[:].rearrange("p (o e) -> p o e", o=1)
                        .broadcast_to((P, EL, E)),
                        ALU.mult)
                    nc.vector.tensor_reduce(
                        dst[:], tmp8[:].rearrange("p (l e) -> p l e", l=EL),
                        axis=AX.X, op=ALU.add)

                # pall[p, s] = 1 iff token p dispatched to slot s
                pall = rt.tile([P, nslot], F32, tag="pall")
                wmask = rt.tile([P, nslot], F32, tag="wmask")
                for l in range(EL):
                    b0, b1 = int(bases[l]), int(bases[l + 1])
                    cb = b1 - b0
                    nc.vector.tensor_tensor(
                        pall[:, b0:b1], c_iota[:, b0:b1],
                        posL[:, l:l + 1].broadcast_to((P, cb)), ALU.is_equal)
                    nc.vector.tensor_tensor(
                        pall[:, b0:b1], pall[:, b0:b1],
                        AL[:, l:l + 1].broadcast_to((P, cb)), ALU.mult)
                    nc.vector.tensor_tensor(
                        wmask[:, b0:b1], pall[:, b0:b1],
                        WL[:, l:l + 1].broadcast_to((P, cb)), ALU.mult)

                tokcol = rt.tile([P, 1], F32, tag="tokcol")
                nc.gpsimd.iota(tokcol[:], pattern=[[0, 1]], base=it * P,
                               channel_multiplier=1,
                               allow_small_or_imprecise_dtypes=True)
                for cb in range(nch):
                    c0 = cb * 512
                    c1 = min(nslot, c0 + 512)
                    nc.tensor.matmul(tbl_ps[:, c0:c1], tokcol[:],
                                     pall[:, c0:c1],
                                     start=(it == 0), stop=(it == NT - 1))
                    nc.tensor.matmul(wks_ps[:, c0:c1], onesc_t[:],
                                     wmask[:, c0:c1],
                                     start=(it == 0), stop=(it == NT - 1))

            tblf = rt.tile([1, nslot], F32, tag="tblf")
            nc.vector.tensor_copy(tblf[:], tbl_ps[:])
            tblu = rt.tile([1, nslot], U32, tag="tblu")
            nc.vector.tensor_copy(tblu[:], tblf[:])
            nc.sync.dma_start(table[:], tblu[:])
            wksf = rt.tile([1, nslot], F32, tag="wksf")
            nc.vector.tensor_copy(wksf[:], wks_ps[:])
            nc.sync.dma_start(wks_d[:], wksf[:])
            # scatter table: padding slots (weight exactly 0) -> sink row T so
            # their in-flight RMW adds never collide with a real token row
            pad = rt.tile([1, nslot], F32, tag="pad")
            nc.vector.tensor_scalar(pad[:], wksf[:], 0.0, None,
                                    op0=ALU.is_equal)
            nc.vector.scalar_tensor_tensor(tblf[:], pad[:], float(T), tblf[:],
                                           op0=ALU.mult, op1=ALU.add)
            tbsu = rt.tile([1, nslot], U32, tag="tbsu")
            nc.vector.tensor_copy(tbsu[:], tblf[:])
            nc.sync.dma_start(table_s[:], tbsu[:])

        # ============ phase S: shared MLP (TP slice SL=256) -> cc_acc init
        with ExitStack() as s_ctx:
            sh_pool = s_ctx.enter_context(tc.tile_pool(name="sh", bufs=3))
            sd_pool = s_ctx.enter_context(tc.tile_pool(name="sd", bufs=1))
            ash_pool = s_ctx.enter_context(tc.tile_pool(name="ash", bufs=2))
            shst_pool = s_ctx.enter_context(tc.tile_pool(name="shst", bufs=4))
            hsh_ps_pool = s_ctx.enter_context(
                tc.tile_pool(name="hshps", bufs=1, space="PSUM"))
            ysh_ps_pool = s_ctx.enter_context(
                tc.tile_pool(name="yshps", bufs=2, space="PSUM"))
            ysb_pool = s_ctx.enter_context(tc.tile_pool(name="ysbs", bufs=3))
            for tcn in range(2):  # halves of T
                hsh_ps = hsh_ps_pool.tile([P, 4 * 512], F32, tag="hsh")
                for kt in range(NKH):
                    sgu_t = sh_pool.tile([P, 2 * SL], BF16, tag="sgu")
                    nc.sync.dma_start(sgu_t[:], sgu_loc[kt * P:(kt + 1) * P, :])
                    hv = shst_pool.tile([P, 512], BF16, tag="hsts")
                    nc.sync.dma_start(
                        hv[:],
                        hsT_bf[kt * P:(kt + 1) * P, tcn * 512:(tcn + 1) * 512])
                    for mt in range(4):
                        nc.tensor.matmul(
                            hsh_ps[:, mt * 512:(mt + 1) * 512],
                            sgu_t[:, mt * P:(mt + 1) * P], hv[:],
                            start=(kt == 0), stop=(kt == NKH - 1))
                # silu(gate)*up: m-tiles 0,1 = gate rows; 2,3 = up rows
                ash_t = ash_pool.tile([P, 2 * 512], BF16, tag="ash")
                for mt in range(2):
                    sil = rt.tile([P, 512], F32, tag="sil")
                    nc.scalar.activation(sil[:],
                                         hsh_ps[:, mt * 512:(mt + 1) * 512],
                                         ACTF.Sigmoid)
                    nc.vector.tensor_tensor(
                        sil[:], sil[:], hsh_ps[:, mt * 512:(mt + 1) * 512],
                        ALU.mult)
                    nc.vector.tensor_tensor(
                        ash_t[:, mt * 512:(mt + 1) * 512], sil[:],
                        hsh_ps[:, (mt + 2) * 512:(mt + 3) * 512], ALU.mult)
                sd_ts = []
                for kt in range(2):
                    sd_t = sd_pool.tile([P, H], BF16, tag=f"sd{kt}")
                    nc.sync.dma_start(sd_t[:], sd_loc[kt * P:(kt + 1) * P, :])
                    sd_ts.append(sd_t)
                for ts in range(4):  # token sub-tiles of 128 in this half
                    tglob = tcn * 4 + ts
                    for hc in range(4):
                        ysh_ps = ysh_ps_pool.tile([P, 512], F32, tag="ysh")
                        for kt in range(2):
                            nc.tensor.matmul(
                                ysh_ps[:],
                                ash_t[:, kt * 512 + ts * P:
                                      kt * 512 + (ts + 1) * P],
                                sd_ts[kt][:, hc * 512:(hc + 1) * 512],
                                start=(kt == 0), stop=(kt == 1))
                        ysb = ysb_pool.tile([P, 512], BF16, tag="ysb_sh")
                        nc.any.tensor_copy(ysb[:], ysh_ps[:])
                        nc.sync.dma_start(
                            cc_acc[tglob * P:(tglob + 1) * P,
                                   hc * 512:(hc + 1) * 512], ysb[:])

        # ============ phase E: expert GEMMs (bf16 x e3m4) + scatter-add
        with ExitStack() as e_ctx:
            xe_pool = e_ctx.enter_context(tc.tile_pool(name="xe", bufs=2))
            xet_pool = e_ctx.enter_context(tc.tile_pool(name="xet", bufs=2))
            idx_pool = e_ctx.enter_context(tc.tile_pool(name="idx", bufs=2))
            gat_pool = e_ctx.enter_context(tc.tile_pool(name="gat", bufs=2))
            at_pool = e_ctx.enter_context(tc.tile_pool(name="at", bufs=2))
            ysb_pool = e_ctx.enter_context(tc.tile_pool(name="ysb", bufs=2))
            gu_ps_pool = e_ctx.enter_context(
                tc.tile_pool(name="gups", bufs=2, space="PSUM"))
            y_ps_pool = e_ctx.enter_context(
                tc.tile_pool(name="yps", bufs=2, space="PSUM"))
            tr_ps_pool = e_ctx.enter_context(
                tc.tile_pool(name="trps", bufs=2, space="PSUM"))

            for l in range(EL):
                nct = int(caps[l]) // P
                b0 = int(bases[l])
                idxs, idxscs, wkts, xets = [], [], [], []
                for ct in range(nct):
                    idxt = idx_pool.tile([P, 1], U32, tag="idxt")
                    nc.sync.dma_start(
                        idxt[:], table[b0 + ct * P: b0 + (ct + 1) * P, :])
                    idxsc = idx_pool.tile([P, 1], U32, tag="idxsc")
                    nc.sync.dma_start(
                        idxsc[:], table_s[b0 + ct * P: b0 + (ct + 1) * P, :])
                    idxscs.append(idxsc)
                    wkt = idx_pool.tile([P, 1], F32, tag="wkt")
                    nc.sync.dma_start(
                        wkt[:], wks_d[b0 + ct * P: b0 + (ct + 1) * P, :])
                    xe_t = xe_pool.tile([P, H], BF16, tag="xe")
                    nc.gpsimd.indirect_dma_start(
                        xe_t[:], None, hs_bf[:],
                        bass.IndirectOffsetOnAxis(ap=idxt[:], axis=0),
                    )
                    xet_t = xet_pool.tile([P, NKH * P], BF16, tag="xet")
                    for kt in range(NKH):
                        tr_ps = tr_ps_pool.tile([P, P], BF16, tag="trps")
                        nc.tensor.transpose(
                            tr_ps[:], xe_t[:, kt * P:(kt + 1) * P], id_t[:])
                        nc.vector.tensor_copy(
                            xet_t[:, kt * P:(kt + 1) * P], tr_ps[:])
                    idxs.append(idxt)
                    wkts.append(wkt)
                    xets.append(xet_t)

                # gate_up: x-stationary, w13 e3m4 moving; psum per (ct, half)
                gus = [gu_ps_pool.tile([P, I], F32, tag="gu", name=f"gu{i}")
                       for i in range(nct)]
                sigs = [gat_pool.tile([P, I], BF16, tag="sig", name=f"sig{i}")
                        for i in range(nct)]
                gates = [gat_pool.tile([P, I], F32, tag="gate",
                                       name=f"gate{i}") for i in range(nct)]
                ats = [at_pool.tile([P, (I // P) * P], BF16, tag="at",
                                    name=f"at{i}") for i in range(nct)]
                for ih in range(2):
                    for kt in range(NKH):
                        w13c = we_pool.tile([P, I], E3M4, tag="w13")
                        nc.sync.dma_start(w13c[:], w13_loc[l, ih, kt])
                        for ct in range(nct):
                            for n in range(2):
                                nc.tensor.matmul(
                                    gus[ct][:, n * 512:(n + 1) * 512],
                                    xets[ct][:, kt * P:(kt + 1) * P],
                                    w13c[:, n * 512:(n + 1) * 512],
                                    start=(kt == 0), stop=(kt == NKH - 1))
                    for ct in range(nct):
                        if ih == 0:
                            nc.scalar.activation(sigs[ct][:], gus[ct][:],
                                                 ACTF.Sigmoid,
                                                 scale=1.0 / W13_SCALE)
                            nc.vector.tensor_copy(gates[ct][:], gus[ct][:])
                        else:
                            # a = gate * up * sigmoid(gate/SCALE)  (scaled)
                            nc.vector.tensor_tensor(gates[ct][:], gates[ct][:],
                                                    gus[ct][:], ALU.mult)
                            a_bf = gat_pool.tile([P, I], BF16, tag="abf")
                            nc.vector.tensor_tensor(a_bf[:], gates[ct][:],
                                                    sigs[ct][:], ALU.mult)
                            # transpose a -> aT
                            for j in range(I // P):
                                tr_ps = tr_ps_pool.tile([P, P], BF16,
                                                        tag="trps")
                                nc.tensor.transpose(
                                    tr_ps[:], a_bf[:, j * P:(j + 1) * P],
                                    id_t[:])
                                nc.vector.tensor_copy(
                                    ats[ct][:, j * P:(j + 1) * P], tr_ps[:])

                # down proj: aT stationary, w2 e3m4 moving; scale by wkt
                ysbs = [ysb_pool.tile([P, H], BF16, tag="ysb",
                                      name=f"ysb{i}") for i in range(nct)]
                for hq in range(4):
                    y_pss = [y_ps_pool.tile([P, 512], F32, tag="yps",
                                             name=f"yps{i}")
                             for i in range(nct)]
                    for j in range(I // P):
                        w2c = w2_pool.tile([P, 512], E3M4, tag="w2")
                        nc.sync.dma_start(w2c[:], w2_loc[l, hq, j])
                        for ct in range(nct):
                            nc.tensor.matmul(
                                y_pss[ct][:], ats[ct][:, j * P:(j + 1) * P],
                                w2c[:], start=(j == 0), stop=(j == I // P - 1))
                    for ct in range(nct):
                        nc.vector.tensor_scalar(
                            ysbs[ct][:, hq * 512:(hq + 1) * 512],
                            y_pss[ct][:], wkts[ct][:], None, op0=ALU.mult)
                # scatter-add this expert's weighted rows into cc_acc.  Rows
                # within one expert are distinct tokens, so the two ct
                # scatters can be in flight together; BETWEEN experts the
                # same token row may appear again, and concurrent RMW adds
                # across SDMA engines lose updates -> fence with a read that
                # forces completion before the next expert's scatters.
                for ct in range(nct):
                    nc.gpsimd.indirect_dma_start(
                        cc_acc[:],
                        bass.IndirectOffsetOnAxis(ap=idxscs[ct][:], axis=0),
                        ysbs[ct][:], None,
                        compute_op=ALU.add,
                    )
                fence = idx_pool.tile([1, P], BF16, tag="fence")
                nc.sync.dma_start(fence[:], cc_acc[0:1, 0:P])

        # ============ cross-core reduce
        if use_collective:
            nc.gpsimd.collective_compute(
                "ReduceScatter", ALU.add,
                replica_groups=[list(range(NCORES))],
                ins=[cc_acc[0:T, :]],
                outs=[cc_out[:]],
            )
            with ExitStack() as o_ctx:
                o_pool = o_ctx.enter_context(tc.tile_pool(name="o", bufs=2))
                for hc in range(2):
                    ob = o_pool.tile([P, 1024], BF16, tag="ob")
                    nc.sync.dma_start(ob[:],
                                      cc_out[:, hc * 1024:(hc + 1) * 1024])
                    of = o_pool.tile([P, 1024], F32, tag="of")
                    nc.vector.tensor_copy(of[:], ob[:])
                    nc.sync.dma_start(out_t[:, hc * 1024:(hc + 1) * 1024],
                                      of[:])
        else:
            with ExitStack() as o_ctx:
                o_pool = o_ctx.enter_context(tc.tile_pool(name="o", bufs=2))
                for it in range(NT):
                    ob = o_pool.tile([P, H], BF16, tag="ob")
                    nc.sync.dma_start(ob[:], cc_acc[it * P:(it + 1) * P, :])
                    of = o_pool.tile([P, H], F32, tag="of")
                    nc.vector.tensor_copy(of[:], ob[:])
                    nc.sync.dma_start(out_t[it * P:(it + 1) * P, :], of[:])

    nc.compile()
    return nc


def _route_counts(hs, gate_w, gate_bias):
    """Host-side replication of the reference routing, to derive per-expert
    loads for capacity assignment (numpy only)."""
    logits = hs.astype(np.float64) @ gate_w.astype(np.float64).T
    scores = 1.0 / (1.0 + np.exp(-logits))
    sc = scores + gate_bias.astype(np.float64)
    grp = sc.reshape(T, G, E // G)
    g2 = np.sort(grp, axis=2)[:, :, -2:].sum(2)
    top_g = np.argsort(-g2, kind="stable", axis=1)[:, :TG]
    gmask = np.zeros((T, G), bool)
    np.put_along_axis(gmask, top_g, True, axis=1)
    masked = np.where(gmask[:, :, None], grp, -np.inf).reshape(T, E)
    topk = np.argsort(-masked, kind="stable", axis=1)[:, :K]
    return np.bincount(topk.reshape(-1), minlength=E)


def make_in_maps(inputs, caps):
    import ml_dtypes
    bf16 = ml_dtypes.bfloat16
    e3m4 = ml_dtypes.float8_e3m4

    hs = np.ascontiguousarray(np.asarray(inputs["hidden_states"], np.float32))
    gate_w = np.asarray(inputs["gate_w"], np.float32)
    gate_bias = np.asarray(inputs["gate_bias"], np.float32)
    w13 = np.asarray(inputs["w13"], np.float32)
    w2 = np.asarray(inputs["w2"], np.float32)
    sgu = np.asarray(inputs["shared_gate_up"], np.float32)
    sd = np.asarray(inputs["shared_down"], np.float32)

    counts = _route_counts(hs, gate_w, gate_bias)
    # big experts (load above the small-capacity tier, with safety margin)
    big_thresh = 112
    order = np.argsort(-counts, kind="stable")
    bigs = [int(e) for e in order if counts[e] > big_thresh]
    smalls = [int(e) for e in order if counts[e] <= big_thresh]
    assert len(bigs) <= NCORES * BIG_PER_CORE, "capacity pattern infeasible"
    # round-robin bigs over cores, then fill with smalls
    assign = [[] for _ in range(NCORES)]
    for i, e in enumerate(bigs):
        assign[i % NCORES].append(e)
    si = 0
    for c in range(NCORES):
        while len(assign[c]) < EL:
            assign[c].append(smalls[si])
            si += 1

    hsT = np.ascontiguousarray(hs.T)
    gwT = np.ascontiguousarray(gate_w.T)
    bias_bc = np.ascontiguousarray(np.broadcast_to(gate_bias, (P, E)))
    u128 = (np.arange(P)[:, None] < np.arange(P)[None, :]).astype(np.float32)
    ident = np.eye(P, dtype=bf16)
    hs_bf = hs.astype(bf16)
    hsT_bf = hsT.astype(bf16)

    def q8(x, scale):
        return np.clip(x * scale, -15.5, 15.5).astype(e3m4)

    in_maps = []
    for c in range(NCORES):
        les = assign[c]
        # w13_loc[l, ih, kt, p, n]: ih 0 = gate cols, 1 = up cols
        w13c = np.stack([w13[e] for e in les])              # [EL, H, 2I]
        w13g = w13c[:, :, :I].reshape(EL, NKH, P, I)
        w13u = w13c[:, :, I:].reshape(EL, NKH, P, I)
        w13p = np.stack([w13g, w13u], axis=1)               # [EL, 2, NKH, P, I]
        # w2_loc[l, hq, j, p, n]
        w2c = np.stack([w2[e] for e in les])                # [EL, I, H]
        w2p = w2c.reshape(EL, I // P, P, 4, 512).transpose(0, 3, 1, 2, 4)
        sgu_c = np.ascontiguousarray(
            np.concatenate([sgu[:, c * SL:(c + 1) * SL],
                            sgu[:, S + c * SL:S + (c + 1) * SL]], axis=1))
        le_bc = np.ascontiguousarray(
            np.broadcast_to(np.asarray(les, np.float32), (P, EL)))
        in_maps.append({
            "hs_bf": hs_bf,
            "hsT": hsT,
            "hsT_bf": hsT_bf,
            "gwT": gwT,
            "bias_bc": bias_bc,
            "u128": u128,
            "ones1": np.ones((1, P), np.float32),
            "onesc": np.ones((P, 1), np.float32),
            "ident_b": ident,
            "le_bc": le_bc,
            "w13_loc": np.ascontiguousarray(q8(w13p, W13_SCALE)),
            "w2_loc": np.ascontiguousarray(q8(w2p, W2_SCALE)),
            "sgu_loc": sgu_c.astype(bf16),
            "sd_loc": np.ascontiguousarray(sd[c * SL:(c + 1) * SL]).astype(
                bf16),
        })
    return in_maps


def kernel(**inputs):
    caps = np.asarray(CAPS, int)
    nc = build_program(caps, use_collective=True)
    in_maps = make_in_maps(inputs, caps)
    res = bass_utils.run_bass_kernel_spmd(nc, in_maps,
                                          core_ids=list(range(NCORES)))
    return np.concatenate([res.results[c]["out"] for c in range(NCORES)],
                          axis=0)
